# revision 1
# baseline (speedup 1.0000x reference)
"""Trainium2 Bass kernel for sparse causal attention (nn_CausalAttentionKV).

Reference computation (fp32, single device):
    q_all = x @ Wq + bq ; k_all = x @ Wk + bk ; v_all = x @ Wv + bv
    q = gather(q_all, query_idx)        # (B, M, D) selected query rows
    att = softmax(mask(q k^T / sqrt(hd)))   # per-query causal mask t <= qidx[m]
    y = (att v) @ Wo + bo

Shapes: B=4, T=4096, D=2048, n_head=16, hd=128, M=512.

Sharding (8 cores): core = 2*b + g  handles batch b and head-group g
(8 heads = 1024 feature cols).  Q/K/V projections are column-parallel,
out-proj is row-parallel; the two partial outputs per batch are summed
on the host.  All matmul inputs are bf16 (fp32 PSUM accumulation).

Host-side prep per core: transpose x/xq to (D, T) layout (the PE needs
the contraction dim on partitions), gather the M query rows of x,
slice/cast weights, precompute the additive causal mask, and compute
per-t-chunk skip bounds from query_idx so fully-masked regions of the
score matrix are never computed (~45% of attention work skipped for
sorted indices; correct for arbitrary indices).

Per-core schedule: Q projection first (small, covers the K/V weight
prefetch), then one fused pass over x computing K^T (head 0 kept in
SBUF, heads 1-7 streamed to a DRAM scratch) and V (resident in SBUF),
then per-head attention with K^T streamed back in (scores, mask and
exp are processed in chunk pairs packed into 2-bank PSUM supertiles to
halve activation-op overhead; softmax normalization is deferred off
the PE critical path), then the output projection.
The attention inner loop runs as a flat (head, batch) software
pipeline: the P@V / row-sum matmuls lag the score/exp stream by one
batch ACROSS head boundaries, so the tensor engine never drains
between heads.  E chunk pairs are pre-summed on the (otherwise idle)
vector engine in bf16 4x mode so the PE runs one softmax row-sum
matmul per pair instead of per chunk.
Measured ~660 us on hardware per NeuronCore (8 cores SPMD), ~93% PE
occupancy, vs a ~650 us bf16-matmul practical floor for this split.
"""

import sys
import types
from contextlib import ExitStack

import numpy as np
import ml_dtypes

import concourse.bass as bass
import concourse.tile as tile
import concourse.mybir as mybir
from concourse import bacc
from concourse.bass_utils import run_bass_kernel_spmd

BF16 = mybir.dt.bfloat16
F32 = mybir.dt.float32
NPBF = ml_dtypes.bfloat16

B, T, D = 4, 4096, 2048
NH, HD, M = 16, 128, 512
NHG = 8            # heads per core (group)
DG = NHG * HD      # 1024 feature cols per core
NT = T // 128      # 32 t-chunks
ND = D // 128      # 16 d-chunks
MASK_VAL = np.float32(-30000.0)


def _install_ntff_hook():
    """Register the axon NTFF profiling hook if the image's antenv lacks it."""
    try:
        from antenv.axon_hooks import get_axon_ntff_profile_hook  # noqa: F401
        return
    except ImportError:
        pass
    try:
        import antenv
        from trn_agent_boot.trn_boot import _ntff_profile_via_ctypes

        mod = types.ModuleType("antenv.axon_hooks")
        hook = [None]
        mod.set_axon_ntff_profile_hook = lambda h: hook.__setitem__(0, h)
        mod.get_axon_ntff_profile_hook = lambda: hook[0]
        sys.modules["antenv.axon_hooks"] = mod
        antenv.axon_hooks = mod
        mod.set_axon_ntff_profile_hook(
            _ntff_profile_via_ctypes("/opt/axon/libaxon_pjrt.so")
        )
    except Exception:
        pass


def build_program(flo, fhi):
    """Build the per-core Bass program.

    flo[i]: first m column with any allowed key in t-chunk i (cols below
            are fully masked there -> never computed).
    fhi[i]: first m column fully allowed in t-chunk i (cols beyond need
            no mask add).
    Both are unions over the 4 batches so one program serves all cores.
    """
    nc = bacc.Bacc("TRN2", target_bir_lowering=False, debug=False)

    xT = nc.dram_tensor("xT", [D, T], BF16, kind="ExternalInput")
    xqT = nc.dram_tensor("xqT", [D, M], BF16, kind="ExternalInput")
    wk = nc.dram_tensor("wk", [D, DG], BF16, kind="ExternalInput")
    wv = nc.dram_tensor("wv", [D, DG], BF16, kind="ExternalInput")
    wq = nc.dram_tensor("wq", [D, DG], BF16, kind="ExternalInput")
    wo = nc.dram_tensor("wo", [DG, D], BF16, kind="ExternalInput")
    maskd = nc.dram_tensor("mask", [T, M], BF16, kind="ExternalInput")
    bks = nc.dram_tensor("bks", [128, NHG], F32, kind="ExternalInput")
    bqs = nc.dram_tensor("bqs", [128, NHG], F32, kind="ExternalInput")
    y = nc.dram_tensor("y", [M, D], F32, kind="ExternalOutput")

    # (c*128+p, t) views for 4-chunk batched DMA
    xTr = xT.rearrange("(c p) t -> p c t", p=128)
    xqTr = xqT.rearrange("(c p) t -> p c t", p=128)
    wkr = wk.rearrange("(c p) t -> p c t", p=128)
    wvr = wv.rearrange("(c p) t -> p c t", p=128)
    wqr = wq.rearrange("(c p) t -> p c t", p=128)
    wor = wo.rearrange("(c p) t -> p c t", p=128)
    maskr = maskd.rearrange("(c p) t -> p c t", p=128)

    with ExitStack() as ctx:
        tc = ctx.enter_context(tile.TileContext(nc))

        # ---- persistent tiles --------------------------------------
        persist = ctx.enter_context(tc.tile_pool(name="persist", bufs=1))
        v_t = [persist.tile([128, DG], BF16, name=f"v{i}", tag=f"v{i}") for i in range(NT)]
        qt_t = [persist.tile([128, M], BF16, name=f"qt{j}", tag=f"qt{j}") for j in range(NHG)]
        ot_t = [persist.tile([128, M], BF16, name=f"ot{j}", tag=f"ot{j}") for j in range(NHG)]
        bias_k = persist.tile([128, NHG], F32, name="bias_k", tag="bias_k")
        bias_q = persist.tile([128, NHG], F32, name="bias_q", tag="bias_q")
        zbias = persist.tile([128, 1], F32, name="zbias", tag="zbias")
        ones_c = persist.tile([128, 1], BF16, name="ones_c", tag="ones_c")
        ones_r = persist.tile([1, 128], F32, name="ones_r", tag="ones_r")
        kt0_sb = persist.tile([128, T], BF16, name="kt0_sb", tag="kt0_sb")
        # mask super-tiles: 4 t-chunks each, shared col-window
        mlo = [min(flo[4 * g : 4 * g + 4]) for g in range(NT // 4)]
        mhi = [max(fhi[4 * g : 4 * g + 4]) for g in range(NT // 4)]
        mask_t = [
            persist.tile(
                [128, 4, max(mhi[g] - mlo[g], 1)], BF16,
                name=f"mask{g}", tag=f"mask{g}",
            )
            for g in range(NT // 4)
        ]
        dram = ctx.enter_context(tc.tile_pool(name="dram", bufs=1, space="DRAM"))
        ktd = dram.tile([NHG, 128, T], BF16, name="ktd")
        ktd_r = ktd.rearrange("j p t -> p j t")

        nc.sync.dma_start(bias_k[:], bks[:])
        nc.sync.dma_start(bias_q[:], bqs[:])
        nc.vector.memset(zbias[:], 0.0)
        nc.vector.memset(ones_c[:], 1.0)
        nc.vector.memset(ones_r[:], 1.0)

        # wk prefetch starts immediately; transfers ride under phase A-Q
        wkp = ctx.enter_context(tc.tile_pool(name="wkp", bufs=1))
        wk_t = [wkp.tile([128, 4, DG], BF16, name=f"wk{d}", tag=f"wk{d}") for d in range(4)]

        # ---- phase A-Q: Qt[j] = ((xq @ wq_j + bq_j)/sqrt(hd))^T ----
        # wq is loaded in per-head column slices so the first matmul
        # group only waits for ~0.5 MB; runs while wk/wv prefetch.
        with (
            nc.named_scope("phase_AQ"),
            tc.tile_pool(name="wqp", bufs=1) as wqp,
            tc.tile_pool(name="xqp", bufs=1) as xqp,
            tc.tile_pool(name="pq", bufs=4, space="PSUM") as pqp,
        ):
            xq_t = [xqp.tile([128, 4, M], BF16, name=f"xq{d}", tag=f"xq{d}") for d in range(4)]
            for d in range(4):
                nc.sync.dma_start(xq_t[d][:], xqTr[:, 4 * d : 4 * d + 4, :])
            # wq in head-pair column slices: (jg, s) -> 4 d-chunks x 256 cols
            wq_t = {}
            for jg in range(4):
                for s in range(4):
                    wq_t[jg, s] = wqp.tile(
                        [128, 4, 256], BF16, name=f"wq{jg}_{s}", tag=f"wq{jg}_{s}"
                    )
                    nc.sync.dma_start(
                        wq_t[jg, s][:],
                        wqr[:, 4 * s : 4 * s + 4, jg * 256 : (jg + 1) * 256],
                    )
            for d in range(4):
                nc.sync.dma_start(wk_t[d][:], wkr[:, 4 * d : 4 * d + 4, :])
            inv_s = 1.0 / float(np.sqrt(HD))
            for j in range(NHG):
                jg, co = j // 2, (j % 2) * 128
                pq = pqp.tile([128, M], F32, name="pq", tag="pq")
                for d in range(ND):
                    nc.tensor.matmul(
                        pq[:],
                        wq_t[jg, d // 4][:, d % 4, co : co + 128],
                        xq_t[d // 4][:, d % 4, :],
                        start=(d == 0),
                        stop=(d == ND - 1),
                    )
                nc.scalar.activation(
                    qt_t[j][:],
                    pq[:],
                    mybir.ActivationFunctionType.Identity,
                    scale=inv_s,
                    bias=bias_q[:, j : j + 1],
                )

        # ---- phase A-KV: one pass over x computing Kt and V --------
        KTS = 512
        with (
            nc.named_scope("phase_AKV"),
            tc.tile_pool(name="wvp", bufs=1) as wvp,
            tc.tile_pool(name="xtp", bufs=2) as xtp,
            tc.tile_pool(name="kst", bufs=3) as kstp,
            tc.tile_pool(name="pk", bufs=3, space="PSUM") as pkp,
            tc.tile_pool(name="pv", bufs=3, space="PSUM") as pvp,
        ):
            wv_t = [wvp.tile([128, 4, DG], BF16, name=f"wv{d}", tag=f"wv{d}") for d in range(4)]
            xt0 = [xtp.tile([128, 4, KTS], BF16, name=f"xt{d}", tag=f"xt{d}") for d in range(4)]
            for d in range(4):
                nc.sync.dma_start(xt0[d][:], xTr[:, 4 * d : 4 * d + 4, 0:KTS])
            for d in range(4):
                nc.sync.dma_start(wv_t[d][:], wvr[:, 4 * d : 4 * d + 4, :])
            for g in range(NT // 4):
                if mlo[g] < M and mhi[g] > mlo[g]:
                    nc.sync.dma_start(
                        mask_t[g][:, :, : mhi[g] - mlo[g]],
                        maskr[:, 4 * g : 4 * g + 4, mlo[g] : mhi[g]],
                    )
            for ts in range(T // KTS):
                if ts == 0:
                    xt_t = xt0
                else:
                    xt_t = [xtp.tile([128, 4, KTS], BF16, name=f"xt{d}", tag=f"xt{d}") for d in range(4)]
                    for d in range(4):
                        nc.sync.dma_start(
                            xt_t[d][:], xTr[:, 4 * d : 4 * d + 4, ts * KTS : (ts + 1) * KTS]
                        )
                # K^T: per head j, (hd, KTS) tile; staged 4 heads per DMA
                for jg in range(2):
                    ks = kstp.tile([128, 4, KTS], BF16, name="ks", tag="ks")
                    for jj in range(4):
                        j = 4 * jg + jj
                        pk = pkp.tile([128, KTS], F32, name="pk", tag="pk")
                        for d in range(ND):
                            nc.tensor.matmul(
                                pk[:],
                                wk_t[d // 4][:, d % 4, j * 128 : (j + 1) * 128],
                                xt_t[d // 4][:, d % 4, :],
                                start=(d == 0),
                                stop=(d == ND - 1),
                            )
                        nc.scalar.activation(
                            kt0_sb[:, ts * KTS : (ts + 1) * KTS] if j == 0
                            else ks[:, jj, :],
                            pk[:],
                            mybir.ActivationFunctionType.Identity,
                            bias=bias_k[:, j : j + 1],
                        )
                    nc.sync.dma_start(
                        ktd_r[:, 4 * jg : 4 * jg + 4, ts * KTS : (ts + 1) * KTS],
                        ks[:],
                    )
                # V: (t, DG) tiles
                for u in range(KTS // 128):
                    i = ts * (KTS // 128) + u
                    for f in range(2):
                        pv = pvp.tile([128, 512], F32, name="pv", tag="pv")
                        for d in range(ND):
                            nc.tensor.matmul(
                                pv[:],
                                xt_t[d // 4][:, d % 4, u * 128 : (u + 1) * 128],
                                wv_t[d // 4][:, d % 4, f * 512 : (f + 1) * 512],
                                start=(d == 0),
                                stop=(d == ND - 1),
                            )
                        nc.vector.tensor_copy(
                            v_t[i][:, f * 512 : (f + 1) * 512], pv[:]
                        )

        # ---- phase B prefetch: out-proj weights + masks ------------
        wop = ctx.enter_context(tc.tile_pool(name="wop", bufs=1))
        wo_t = [wop.tile([128, 4, D], BF16, name=f"wo{d}", tag=f"wo{d}") for d in range(2)]
        for d in range(2):
            nc.sync.dma_start(wo_t[d][:], wor[:, 4 * d : 4 * d + 4, :])

        # ---- phase B: attention per head, 4-chunk batched ----------
        chunks = [i for i in range(NT) if flo[i] < M]
        pairs = [chunks[k : k + 2] for k in range(0, len(chunks), 2)]
        batches = [pairs[k : k + 2] for k in range(0, len(pairs), 2)]
        with (
            nc.named_scope("phase_B"),
            tc.tile_pool(name="kth", bufs=2) as kthp,
            tc.tile_pool(name="ps", bufs=2, space="PSUM") as psp,
            tc.tile_pool(name="po", bufs=2, space="PSUM") as pop,
            tc.tile_pool(name="pl", bufs=2, space="PSUM") as plp,
            tc.tile_pool(name="esb", bufs=5) as esb,
            tc.tile_pool(name="lsb", bufs=2) as lsb,
        ):
            po_q, pl_q = {}, {}

            def emit_norm(j):
                """Normalize head j: ot[j] = po[j] / l[j] (off PE critical path).

                l is broadcast across partitions on the PE first, so the
                reciprocal runs 128 lanes wide instead of on 1 partition.
                """
                po, pl = po_q.pop(j), pl_q.pop(j)
                l_sb = lsb.tile([1, M], F32, name="l", tag="l")
                linv = lsb.tile([1, M], F32, name="linv", tag="linv")
                nc.vector.tensor_copy(l_sb[:], pl[:])
                nc.vector.reciprocal_approx_fast(linv[:], l_sb[:])
                pb = psp.tile([128, M], F32, name="pb", tag="ps")
                nc.tensor.matmul(pb[:], ones_r[:], linv[:], start=True, stop=True)
                lb_sb = lsb.tile([128, M], F32, name="lb", tag="lb")
                nc.scalar.copy(lb_sb[:], pb[:])
                nc.vector.tensor_mul(ot_t[j][:], po[:], lb_sb[:])

            kth = {0: kt0_sb}
            state = {}  # j -> [po_start_pending, l_start_pending]

            def drain(pj, cur, last_b):
                """Emit the lagged PV + row-sum matmuls for head pj's batch."""
                st = state[pj]
                for k, (pair, e, esum, lo) in enumerate(cur):
                    for u, i in enumerate(pair):
                        nc.tensor.matmul(
                            po_q[pj][:, lo:M],
                            v_t[i][:, pj * 128 : (pj + 1) * 128],
                            e[:, u, lo:M],
                            start=st[0],
                            stop=(last_b and k == len(cur) - 1 and u == len(pair) - 1),
                            skip_group_check=True,
                        )
                        st[0] = False
                for k, (pair, e, esum, lo) in enumerate(cur):
                    rs = esum[:, lo:M] if esum is not None else e[:, 0, lo:M]
                    nc.tensor.matmul(
                        pl_q[pj][:, lo:M], ones_c[:], rs,
                        start=st[1], stop=(last_b and k == len(cur) - 1),
                        skip_group_check=True,
                    )
                    st[1] = False

            # flat (head, batch) pipeline: PV/l lag the S/exp stream by one
            # batch ACROSS head boundaries, so the PE never drains between
            # heads
            pend = None  # (j, cur, is_last_batch_of_head)
            for j in range(NHG):
                po_q[j] = pop.tile([128, M], F32, name="po", tag="po")
                pl_q[j] = plp.tile([1, M], F32, name="pl", tag="pl")
                state[j] = [True, True]
                for bi, batch in enumerate(batches):
                    cur = []
                    for pair in batch:
                        lo_min = min(flo[i] for i in pair)
                        pst = psp.tile([128, 2, M], F32, name="pst", tag="ps")
                        for u, i in enumerate(pair):
                            nc.tensor.matmul(
                                pst[:, u, lo_min:M],
                                kth[j][:, i * 128 : (i + 1) * 128],
                                qt_t[j][:, lo_min:M],
                                start=True,
                                stop=True,
                                skip_group_check=True,
                            )
                        # one mask add + one exp per pair; mask cols beyond a
                        # chunk's own [lo, fhi) window add 0 or touch lanes
                        # the narrower chunk never reads
                        fhi_max = max(fhi[i] for i in pair)
                        g = pair[0] // 4
                        um = pair[0] % 4
                        if lo_min < fhi_max:
                            nc.vector.tensor_add(
                                pst[:, : len(pair), lo_min:fhi_max],
                                pst[:, : len(pair), lo_min:fhi_max],
                                mask_t[g][:, um : um + len(pair), lo_min - mlo[g] : fhi_max - mlo[g]],
                            )
                        e = esb.tile([128, 2, M], BF16, name="e", tag="e")
                        nc.scalar.activation(
                            e[:, : len(pair), lo_min:M],
                            pst[:, : len(pair), lo_min:M],
                            mybir.ActivationFunctionType.Exp,
                            bias=zbias[:],
                        )
                        if len(pair) == 2:
                            # pair-sum on DVE (bf16 4x) so the PE does one
                            # row-sum matmul per pair instead of per chunk
                            esum = esb.tile([128, M], BF16, name="esum", tag="esum", bufs=3)
                            nc.vector.tensor_add(
                                esum[:, lo_min:M],
                                e[:, 0, lo_min:M],
                                e[:, 1, lo_min:M],
                            )
                            cur.append((pair, e, esum, lo_min))
                        else:
                            cur.append((pair, e, None, lo_min))
                    if pend is not None:
                        drain(*pend)
                    pend = (j, cur, bi == len(batches) - 1)
                    if bi == 1:
                        # prefetch next head's K^T; emit previous head's norm
                        if j + 1 < NHG:
                            kth[j + 1] = kthp.tile([128, T], BF16, name="kth", tag="kth")
                            nc.sync.dma_start(kth[j + 1][:], ktd[j + 1])
                    if bi == 2:
                        if j > 0 and (j - 1) in po_q:
                            emit_norm(j - 1)
            drain(*pend)
            emit_norm(NHG - 1)

        # ---- phase C: y = O @ wo  (row-parallel partial) -----------
        with (
            nc.named_scope("phase_C"),
            tc.tile_pool(name="py", bufs=2, space="PSUM") as pyp,
            tc.tile_pool(name="ysb", bufs=3) as ysb,
        ):
            # fo pairs share the stationary ot slice -> one weight load
            # feeds two 512-wide matmuls
            for mb in range(M // 128):
                for fp in range(D // 1024):
                    py = [
                        pyp.tile([128, 512], F32, name="py", tag=f"py{h}")
                        for h in range(2)
                    ]
                    for j in range(NHG):
                        for h in range(2):
                            fo = 2 * fp + h
                            nc.tensor.matmul(
                                py[h][:],
                                ot_t[j][:, mb * 128 : (mb + 1) * 128],
                                wo_t[j // 4][:, j % 4, fo * 512 : (fo + 1) * 512],
                                start=(j == 0),
                                stop=(j == NHG - 1),
                                skip_group_check=True,
                            )
                    for h in range(2):
                        ys = ysb.tile([128, 512], F32, name="ys", tag="ys")
                        nc.scalar.copy(ys[:], py[h][:])
                        nc.sync.dma_start(
                            y[
                                mb * 128 : (mb + 1) * 128,
                                (2 * fp + h) * 512 : (2 * fp + h + 1) * 512,
                            ],
                            ys[:],
                        )

    nc.compile()
    return nc


_cache = {}


def _get_program(flo, fhi):
    key = (tuple(flo), tuple(fhi))
    if key not in _cache:
        _cache[key] = build_program(list(flo), list(fhi))
    return _cache[key]


def _prep(inputs):
    x = np.asarray(inputs["x"], dtype=np.float32)
    qidx = np.asarray(inputs["query_idx"]).astype(np.int64)
    Wq = np.asarray(inputs["Wq"], dtype=np.float32)
    Wk = np.asarray(inputs["Wk"], dtype=np.float32)
    Wv = np.asarray(inputs["Wv"], dtype=np.float32)
    Wo = np.asarray(inputs["Wo"], dtype=np.float32)
    bq = np.asarray(inputs["bq"], dtype=np.float32)
    bk = np.asarray(inputs["bk"], dtype=np.float32)
    bv = np.asarray(inputs["bv"], dtype=np.float32)
    bo = np.asarray(inputs["bo"], dtype=np.float32)

    # Per-t-chunk skip bounds, union over batches.  flo[i] = first m that
    # attends into chunk i (everything below is fully masked there);
    # fhi[i] = one past the last m only partially covered by chunk i.
    # Computed positionally so they are correct even for unsorted
    # query_idx (just less effective at skipping).
    flo = [M] * NT
    fhi = [0] * NT
    for b in range(B):
        for i in range(NT):
            allowed = qidx[b] >= 128 * i          # chunk i not fully masked
            partial = qidx[b] < 128 * (i + 1)     # chunk i not fully allowed
            lo_b = int(np.argmax(allowed)) if allowed.any() else M
            hi_b = M - int(np.argmax(partial[::-1])) if partial.any() else 0
            flo[i] = min(flo[i], lo_b)
            fhi[i] = max(fhi[i], hi_b)

    in_maps = []
    tgrid = np.arange(T)[:, None]
    for core in range(8):
        b, g = divmod(core, 2)
        sl = slice(g * DG, (g + 1) * DG)
        xb = x[b]
        mask = np.where(tgrid <= qidx[b][None, :], np.float32(0), MASK_VAL)
        in_maps.append(
            {
                "xT": np.ascontiguousarray(xb.T.astype(NPBF)),
                "xqT": np.ascontiguousarray(xb[qidx[b]].T.astype(NPBF)),
                "wk": np.ascontiguousarray(Wk[:, sl].astype(NPBF)),
                "wv": np.ascontiguousarray(Wv[:, sl].astype(NPBF)),
                "wq": np.ascontiguousarray(Wq[:, sl].astype(NPBF)),
                "wo": np.ascontiguousarray(Wo[sl, :].astype(NPBF)),
                "mask": np.ascontiguousarray(mask.astype(NPBF)),
                "bks": np.ascontiguousarray(bk[sl].reshape(NHG, 128).T),
                "bqs": np.ascontiguousarray(
                    (bq[sl] / np.sqrt(HD)).reshape(NHG, 128).T.astype(np.float32)
                ),
            }
        )

    const = (bv.astype(np.float64) @ Wo.astype(np.float64) + bo).astype(np.float32)
    return flo, fhi, in_maps, const


def run(inputs, trace=False, trace_kwargs=None):
    _install_ntff_hook()
    flo, fhi, in_maps, const = _prep(inputs)
    nc = _get_program(flo, fhi)
    res = run_bass_kernel_spmd(
        nc, in_maps, list(range(8)), trace=trace, **(trace_kwargs or {})
    )
    out = np.zeros((B, M, D), dtype=np.float32)
    for b in range(B):
        out[b] = res.results[2 * b]["y"] + res.results[2 * b + 1]["y"] + const
    return out, res


def kernel(**inputs) -> np.ndarray:
    out, _ = run(inputs, trace=False)
    return out



# revision 3
# speedup vs baseline: 1.0137x; 1.0137x over previous
"""Trainium2 Bass kernel for sparse causal attention (nn_CausalAttentionKV).

Reference computation (fp32, single device):
    q_all = x @ Wq + bq ; k_all = x @ Wk + bk ; v_all = x @ Wv + bv
    q = gather(q_all, query_idx)        # (B, M, D) selected query rows
    att = softmax(mask(q k^T / sqrt(hd)))   # per-query causal mask t <= qidx[m]
    y = (att v) @ Wo + bo

Shapes: B=4, T=4096, D=2048, n_head=16, hd=128, M=512.

Sharding (8 cores): core = 2*b + g  handles batch b and head-group g
(8 heads = 1024 feature cols).  Q/K/V projections are column-parallel,
out-proj is row-parallel; the two partial outputs per batch are summed
on the host.  All matmul inputs are bf16 (fp32 PSUM accumulation).

FUSED design (v2): attention is fused into the K/V projection pass,
flash-attention style.  For each 512-key window ts: project K (8 heads)
and V, then immediately compute scores, mask, exp and P@V for all heads
on those keys.  This hides the ~93us of scalar-engine exp work (which
previously bounded a separate attention phase) under the projection
matmul wall, eliminates the K^T DRAM round trip, and turns V into a
small SBUF ring instead of an 8 MB resident tensor.

Softmax bookkeeping avoids PE work: exp outputs accumulate into a
per-head fp32 e_total on the vector engine; ONE row-sum matmul per
head (vs one per chunk-pair) yields l; 1/l is broadcast across
partitions with a cheap bf16 rank-1 matmul (vs fp32 LOW_HIGH passes).
The un-normalized P@V partials accumulate into po_sb on the vector
engine (PSUM holds only one window's partial), and normalization
multiplies once per head at the end.

The Q projection is pipelined by d-chunk (8 PSUM banks accumulate all
8 heads while wq/xq stream in), so the PE starts ~2us after launch
instead of waiting ~12us for the full Q working set.
"""

import sys
import types
from contextlib import ExitStack

import numpy as np
import ml_dtypes

import concourse.bass as bass
import concourse.tile as tile
import concourse.mybir as mybir
from concourse import bacc
from concourse.bass_utils import run_bass_kernel_spmd

BF16 = mybir.dt.bfloat16
F32 = mybir.dt.float32
NPBF = ml_dtypes.bfloat16

B, T, D = 4, 4096, 2048
NH, HD, M = 16, 128, 512
NHG = 8            # heads per core (group)
DG = NHG * HD      # 1024 feature cols per core
NT = T // 128      # 32 t-chunks
ND = D // 128      # 16 d-chunks
KTS = 512          # keys per fused iteration
NTS = T // KTS     # 8 fused iterations
MASK_VAL = np.float32(-30000.0)


def _install_ntff_hook():
    """Register the axon NTFF profiling hook if the image's antenv lacks it."""
    try:
        from antenv.axon_hooks import get_axon_ntff_profile_hook  # noqa: F401
        return
    except ImportError:
        pass
    try:
        import antenv
        from trn_agent_boot.trn_boot import _ntff_profile_via_ctypes

        mod = types.ModuleType("antenv.axon_hooks")
        hook = [None]
        mod.set_axon_ntff_profile_hook = lambda h: hook.__setitem__(0, h)
        mod.get_axon_ntff_profile_hook = lambda: hook[0]
        sys.modules["antenv.axon_hooks"] = mod
        antenv.axon_hooks = mod
        mod.set_axon_ntff_profile_hook(
            _ntff_profile_via_ctypes("/opt/axon/libaxon_pjrt.so")
        )
    except Exception:
        pass


def build_program(flo, fhi):
    """Build the per-core Bass program.

    flo[i]: first m column with any allowed key in t-chunk i (cols below
    are fully masked there -> never computed).
    fhi[i]: first m column fully allowed in t-chunk i (cols beyond need
    no mask add).  Both are unions over the 4 batches so one program
    serves all cores.  flo is nondecreasing (qidx sorted per batch).
    """
    nc = bacc.Bacc("TRN2", target_bir_lowering=False, debug=False)

    xT = nc.dram_tensor("xT", [D, T], BF16, kind="ExternalInput")
    xqT = nc.dram_tensor("xqT", [D, M], BF16, kind="ExternalInput")
    wk = nc.dram_tensor("wk", [D, DG], BF16, kind="ExternalInput")
    wv = nc.dram_tensor("wv", [D, DG], BF16, kind="ExternalInput")
    wq = nc.dram_tensor("wq", [D, DG], BF16, kind="ExternalInput")
    wo = nc.dram_tensor("wo", [DG, D], BF16, kind="ExternalInput")
    maskd = nc.dram_tensor("mask", [T, M], BF16, kind="ExternalInput")
    bks = nc.dram_tensor("bks", [128, NHG], F32, kind="ExternalInput")
    bqs = nc.dram_tensor("bqs", [128, NHG], F32, kind="ExternalInput")
    y = nc.dram_tensor("y", [M, D], F32, kind="ExternalOutput")

    # (c*128+p, t) views for chunked DMA
    xTr = xT.rearrange("(c p) t -> p c t", p=128)
    xqTr = xqT.rearrange("(c p) t -> p c t", p=128)
    wkr = wk.rearrange("(c p) t -> p c t", p=128)
    wvr = wv.rearrange("(c p) t -> p c t", p=128)
    wqr = wq.rearrange("(c p) t -> p c t", p=128)
    wor = wo.rearrange("(c p) t -> p c t", p=128)
    maskr = maskd.rearrange("(c p) t -> p c t", p=128)

    # active chunks per ts window (flo nondecreasing -> consecutive prefix)
    def win_chunks(ts):
        return [i for i in range(4 * ts, 4 * ts + 4) if flo[i] < M]

    with ExitStack() as ctx:
        tc = ctx.enter_context(tile.TileContext(nc))

        # ---- persistent tiles --------------------------------------
        persist = ctx.enter_context(tc.tile_pool(name="persist", bufs=1))
        qt_t = [persist.tile([128, M], BF16, name=f"qt{j}", tag=f"qt{j}") for j in range(NHG)]
        ot_t = [persist.tile([128, M], BF16, name=f"ot{j}", tag=f"ot{j}") for j in range(NHG)]
        etot = [persist.tile([128, M], F32, name=f"et{j}", tag=f"et{j}") for j in range(NHG)]
        po_sb = [persist.tile([128, M], F32, name=f"po{j}", tag=f"po{j}") for j in range(NHG)]
        bias_k = persist.tile([128, NHG], F32, name="bias_k", tag="bias_k")
        bias_q = persist.tile([128, NHG], F32, name="bias_q", tag="bias_q")
        zbias = persist.tile([128, 1], F32, name="zbias", tag="zbias")
        ones_c = persist.tile([128, 1], BF16, name="ones_c", tag="ones_c")
        ones_r = persist.tile([1, 128], BF16, name="ones_r", tag="ones_r")
        # mask super-tiles: 4 t-chunks each (== one ts window), shared col-window
        mlo = [min(flo[4 * g : 4 * g + 4]) for g in range(NTS)]
        mhi = [max(fhi[4 * g : 4 * g + 4]) for g in range(NTS)]
        mask_t = [
            persist.tile(
                [128, 4, max(mhi[g] - mlo[g], 1)], BF16,
                name=f"mask{g}", tag=f"mask{g}",
            )
            for g in range(NTS)
        ]

        nc.sync.dma_start(bias_k[:], bks[:])
        nc.sync.dma_start(bias_q[:], bqs[:])
        nc.vector.memset(zbias[:], 0.0)
        nc.vector.memset(ones_c[:], 1.0)
        nc.vector.memset(ones_r[:], 1.0)

        # ---- phase AQ: Qt[j] = ((xq @ wq_j + bq_j)/sqrt(hd))^T -----
        # d-chunk pipelined: all 8 heads accumulate in 8 PSUM banks while
        # wq/xq stream in 0.375 MB per stage -> PE starts ~2us in.
        with (
            nc.named_scope("phase_AQ"),
            tc.tile_pool(name="wqp", bufs=2) as wqp,
            tc.tile_pool(name="xqp", bufs=2) as xqp,
            tc.tile_pool(name="pq", bufs=1, space="PSUM") as pqp,
        ):
            pq = [pqp.tile([128, M], F32, name=f"pq{j}", tag=f"pq{j}") for j in range(NHG)]
            for d in range(ND):
                wq_d = wqp.tile([128, DG], BF16, name="wqd", tag="wqd")
                nc.sync.dma_start(wq_d[:], wqr[:, d, :])
                xq_d = xqp.tile([128, M], BF16, name="xqd", tag="xqd")
                nc.sync.dma_start(xq_d[:], xqTr[:, d, :])
                for j in range(NHG):
                    nc.tensor.matmul(
                        pq[j][:],
                        wq_d[:, j * 128 : (j + 1) * 128],
                        xq_d[:],
                        start=(d == 0),
                        stop=(d == ND - 1),
                        skip_group_check=True,
                    )
            inv_s = 1.0 / float(np.sqrt(HD))
            for j in range(NHG):
                nc.scalar.activation(
                    qt_t[j][:],
                    pq[j][:],
                    mybir.ActivationFunctionType.Identity,
                    scale=inv_s,
                    bias=bias_q[:, j : j + 1],
                )

        # ---- weight/mask prefetch for the fused pass ---------------
        wkp = ctx.enter_context(tc.tile_pool(name="wkp", bufs=1))
        wvp = ctx.enter_context(tc.tile_pool(name="wvp", bufs=1))
        wk_t = [wkp.tile([128, 4, DG], BF16, name=f"wk{d}", tag=f"wk{d}") for d in range(4)]
        wv_t = [wvp.tile([128, 4, DG], BF16, name=f"wv{d}", tag=f"wv{d}") for d in range(4)]
        for d in range(4):
            nc.sync.dma_start(wk_t[d][:], wkr[:, 4 * d : 4 * d + 4, :])
        for g in range(NTS):
            if mlo[g] < M and mhi[g] > mlo[g]:
                nc.sync.dma_start(
                    mask_t[g][:, :, : mhi[g] - mlo[g]],
                    maskr[:, 4 * g : 4 * g + 4, mlo[g] : mhi[g]],
                )
        for d in range(4):
            nc.sync.dma_start(wv_t[d][:], wvr[:, 4 * d : 4 * d + 4, :])

        # ---- fused pass: K/V projection + attention per ts window --
        with (
            nc.named_scope("phase_F"),
            tc.tile_pool(name="xtp", bufs=2) as xtp,
            tc.tile_pool(name="ktp", bufs=4) as ktp,
            tc.tile_pool(name="vtp", bufs=8) as vtp,
            tc.tile_pool(name="esb", bufs=14) as esb,
            tc.tile_pool(name="kv", bufs=2, space="PSUM") as kvp,
            tc.tile_pool(name="ps", bufs=5, space="PSUM") as psp,
            tc.tile_pool(name="po", bufs=1, space="PSUM") as pop,
        ):
            et_started = [False] * NHG   # etot[j] initialized?
            po_started = [False] * NHG   # po_sb[j] initialized?
            pend = []                    # deferred P@V work items

            def emit_scores(j, ts, kt, chunks):
                """Scores+mask+exp for head j on window ts; per-chunk tiles."""
                work = []
                for i in chunks:
                    lo, hi = flo[i], fhi[i]
                    u = i % 4
                    pst = psp.tile([128, M], F32, name="pst", tag="ps")
                    nc.tensor.matmul(
                        pst[:, lo:M],
                        kt[:, u * 128 : (u + 1) * 128],
                        qt_t[j][:, lo:M],
                        start=True,
                        stop=True,
                        skip_group_check=True,
                    )
                    if lo < hi:
                        nc.vector.tensor_add(
                            pst[:, lo:hi],
                            pst[:, lo:hi],
                            mask_t[ts][:, u, lo - mlo[ts] : hi - mlo[ts]],
                        )
                    e = esb.tile([128, M], BF16, name="e", tag="e")
                    nc.scalar.activation(
                        e[:, lo:M],
                        pst[:, lo:M],
                        mybir.ActivationFunctionType.Exp,
                        bias=zbias[:],
                    )
                    # accumulate softmax denominator on the DVE
                    if not et_started[j]:
                        nc.vector.tensor_copy(etot[j][:, lo:M], e[:, lo:M])
                        if lo > 0:
                            nc.vector.memset(etot[j][:, 0:lo], 0.0)
                        et_started[j] = True
                    else:
                        nc.vector.tensor_add(
                            etot[j][:, lo:M], etot[j][:, lo:M], e[:, lo:M]
                        )
                    work.append((i, e, lo))
                return work

            def emit_pv(item):
                """P@V for head j window ts into PSUM, then DVE-accumulate.

                PV matmuls run in chunk order (lo nondecreasing), so the
                start=True region [lo0:M] covers every later chunk's
                [lo_i:M] and no PSUM region is read uninitialized.
                """
                j, work, vts = item
                lo0 = work[0][2]
                pot = pop.tile([128, M], F32, name="pot", tag="po")
                for k, (i, e, lo) in enumerate(work):
                    nc.tensor.matmul(
                        pot[:, lo:M],
                        vts[i % 4][:, j * 128 : (j + 1) * 128],
                        e[:, lo:M],
                        start=(k == 0),
                        stop=(k == len(work) - 1),
                        skip_group_check=True,
                    )
                if not po_started[j]:
                    nc.vector.tensor_copy(po_sb[j][:, lo0:M], pot[:, lo0:M])
                    if lo0 > 0:
                        nc.vector.memset(po_sb[j][:, 0:lo0], 0.0)
                    po_started[j] = True
                else:
                    nc.vector.tensor_add(
                        po_sb[j][:, lo0:M], po_sb[j][:, lo0:M], pot[:, lo0:M]
                    )

            for ts in range(NTS):
                xt_t = [xtp.tile([128, 4, KTS], BF16, name=f"xt{d}", tag=f"xt{d}") for d in range(4)]
                for d in range(4):
                    nc.sync.dma_start(
                        xt_t[d][:], xTr[:, 4 * d : 4 * d + 4, ts * KTS : (ts + 1) * KTS]
                    )
                chunks = win_chunks(ts)
                # K projection, one head at a time; scores chase the K
                # stream and older windows' P@V drains between groups.
                kts = {}
                sc_q = []
                for j in range(NHG):
                    pk = kvp.tile([128, KTS], F32, name="pk", tag="kv")
                    for d in range(ND):
                        nc.tensor.matmul(
                            pk[:],
                            wk_t[d // 4][:, d % 4, j * 128 : (j + 1) * 128],
                            xt_t[d // 4][:, d % 4, :],
                            start=(d == 0),
                            stop=(d == ND - 1),
                        )
                    kt = ktp.tile([128, KTS], BF16, name="kt", tag="kt")
                    nc.scalar.activation(
                        kt[:], pk[:],
                        mybir.ActivationFunctionType.Identity,
                        bias=bias_k[:, j : j + 1],
                    )
                    kts[j] = kt
                    if j >= 1 and chunks:
                        sc_q.append((j - 1, emit_scores(j - 1, ts, kts.pop(j - 1), chunks)))
                        if len(pend) >= 2:
                            emit_pv(pend.pop(0))
                # V projection: 4 v tiles [t=128, DG] per window
                vts = []
                for u in range(4):
                    vt = vtp.tile([128, DG], BF16, name="vt", tag="vt")
                    for f in range(2):
                        pv = kvp.tile([128, 512], F32, name="pv", tag="kv")
                        for d in range(ND):
                            nc.tensor.matmul(
                                pv[:],
                                xt_t[d // 4][:, d % 4, u * 128 : (u + 1) * 128],
                                wv_t[d // 4][:, d % 4, f * 512 : (f + 1) * 512],
                                start=(d == 0),
                                stop=(d == ND - 1),
                            )
                        nc.vector.tensor_copy(vt[:, f * 512 : (f + 1) * 512], pv[:])
                    vts.append(vt)
                    if u == 1 and chunks:
                        # last head's scores ride behind the V stream
                        sc_q.append((NHG - 1, emit_scores(NHG - 1, ts, kts.pop(NHG - 1), chunks)))
                # queue this window's P@V now that its v tiles exist
                for j, work in sc_q:
                    pend.append((j, work, vts))
                while len(pend) > 6:
                    emit_pv(pend.pop(0))
            while pend:
                emit_pv(pend.pop(0))

        # ---- out-proj weights (loaded after fused pools free) ------
        wop = ctx.enter_context(tc.tile_pool(name="wop", bufs=1))
        wo_t = [wop.tile([128, 4, D], BF16, name=f"wo{d}", tag=f"wo{d}") for d in range(2)]
        for d in range(2):
            nc.sync.dma_start(wo_t[d][:], wor[:, 4 * d : 4 * d + 4, :])

        # ---- tail: l = rowsum(etot); ot[j] = po_sb[j] / l ----------
        # ---- phase C: y = O @ wo  (row-parallel partial) -----------
        with (
            nc.named_scope("phase_NC"),
            tc.tile_pool(name="ebf", bufs=2) as ebfp,
            tc.tile_pool(name="lsb", bufs=2) as lsbp,
            tc.tile_pool(name="lbp", bufs=2) as lbp,
            tc.tile_pool(name="pl", bufs=2, space="PSUM") as plp,
            tc.tile_pool(name="pb", bufs=2, space="PSUM") as pbp,
            tc.tile_pool(name="py", bufs=2, space="PSUM") as pyp,
            tc.tile_pool(name="ysb", bufs=3) as ysb,
        ):
            for j in range(NHG):
                ebf = ebfp.tile([128, M], BF16, name="ebf", tag="ebf")
                nc.vector.tensor_copy(ebf[:], etot[j][:])
                pl = plp.tile([1, M], F32, name="pl", tag="pl")
                nc.tensor.matmul(pl[:], ones_c[:], ebf[:], start=True, stop=True)
                l_sb = lsbp.tile([1, M], F32, name="l", tag="l")
                linv = lsbp.tile([1, M], F32, name="linv", tag="linv")
                linb = lsbp.tile([1, M], BF16, name="linb", tag="linb")
                nc.vector.tensor_copy(l_sb[:], pl[:])
                nc.vector.reciprocal_approx_fast(linv[:], l_sb[:])
                nc.vector.tensor_copy(linb[:], linv[:])
                pb = pbp.tile([128, M], F32, name="pb", tag="pb")
                nc.tensor.matmul(pb[:], ones_r[:], linb[:], start=True, stop=True)
                lb = lbp.tile([128, M], BF16, name="lb", tag="lb")
                nc.scalar.copy(lb[:], pb[:])
                nc.vector.tensor_mul(ot_t[j][:], po_sb[j][:], lb[:])

            # fo pairs share the stationary ot slice -> one weight load
            # feeds two 512-wide matmuls
            for mb in range(M // 128):
                for fp in range(D // 1024):
                    py = [
                        pyp.tile([128, 512], F32, name="py", tag=f"py{h}")
                        for h in range(2)
                    ]
                    for j in range(NHG):
                        for h in range(2):
                            fo = 2 * fp + h
                            nc.tensor.matmul(
                                py[h][:],
                                ot_t[j][:, mb * 128 : (mb + 1) * 128],
                                wo_t[j // 4][:, j % 4, fo * 512 : (fo + 1) * 512],
                                start=(j == 0),
                                stop=(j == NHG - 1),
                                skip_group_check=True,
                            )
                    for h in range(2):
                        ys = ysb.tile([128, 512], F32, name="ys", tag="ys")
                        nc.scalar.copy(ys[:], py[h][:])
                        nc.sync.dma_start(
                            y[
                                mb * 128 : (mb + 1) * 128,
                                (2 * fp + h) * 512 : (2 * fp + h + 1) * 512,
                            ],
                            ys[:],
                        )

    nc.compile()
    return nc


_cache = {}


def _get_program(flo, fhi):
    key = (tuple(flo), tuple(fhi))
    if key not in _cache:
        _cache[key] = build_program(list(flo), list(fhi))
    return _cache[key]


def _prep(inputs):
    x = np.asarray(inputs["x"], dtype=np.float32)
    qidx = np.asarray(inputs["query_idx"]).astype(np.int64)
    Wq = np.asarray(inputs["Wq"], dtype=np.float32)
    Wk = np.asarray(inputs["Wk"], dtype=np.float32)
    Wv = np.asarray(inputs["Wv"], dtype=np.float32)
    Wo = np.asarray(inputs["Wo"], dtype=np.float32)
    bq = np.asarray(inputs["bq"], dtype=np.float32)
    bk = np.asarray(inputs["bk"], dtype=np.float32)
    bv = np.asarray(inputs["bv"], dtype=np.float32)
    bo = np.asarray(inputs["bo"], dtype=np.float32)

    # Per-t-chunk skip bounds, union over batches.  flo[i] = first m that
    # attends into chunk i (everything below is fully masked there);
    # fhi[i] = one past the last m only partially covered by chunk i.
    # Computed positionally so they are correct even for unsorted
    # query_idx (just less effective at skipping).
    flo = [M] * NT
    fhi = [0] * NT
    for b in range(B):
        for i in range(NT):
            allowed = qidx[b] >= 128 * i          # chunk i not fully masked
            partial = qidx[b] < 128 * (i + 1)     # chunk i not fully allowed
            lo_b = int(np.argmax(allowed)) if allowed.any() else M
            hi_b = M - int(np.argmax(partial[::-1])) if partial.any() else 0
            flo[i] = min(flo[i], lo_b)
            fhi[i] = max(fhi[i], hi_b)

    in_maps = []
    tgrid = np.arange(T)[:, None]
    for core in range(8):
        b, g = divmod(core, 2)
        sl = slice(g * DG, (g + 1) * DG)
        xb = x[b]
        mask = np.where(tgrid <= qidx[b][None, :], np.float32(0), MASK_VAL)
        in_maps.append(
            {
                "xT": np.ascontiguousarray(xb.T.astype(NPBF)),
                "xqT": np.ascontiguousarray(xb[qidx[b]].T.astype(NPBF)),
                "wk": np.ascontiguousarray(Wk[:, sl].astype(NPBF)),
                "wv": np.ascontiguousarray(Wv[:, sl].astype(NPBF)),
                "wq": np.ascontiguousarray(Wq[:, sl].astype(NPBF)),
                "wo": np.ascontiguousarray(Wo[sl, :].astype(NPBF)),
                "mask": np.ascontiguousarray(mask.astype(NPBF)),
                "bks": np.ascontiguousarray(bk[sl].reshape(NHG, 128).T),
                "bqs": np.ascontiguousarray(
                    (bq[sl] / np.sqrt(HD)).reshape(NHG, 128).T.astype(np.float32)
                ),
            }
        )

    const = (bv.astype(np.float64) @ Wo.astype(np.float64) + bo).astype(np.float32)
    return flo, fhi, in_maps, const


def run(inputs, trace=False, trace_kwargs=None):
    _install_ntff_hook()
    flo, fhi, in_maps, const = _prep(inputs)
    nc = _get_program(flo, fhi)
    res = run_bass_kernel_spmd(
        nc, in_maps, list(range(8)), trace=trace, **(trace_kwargs or {})
    )
    out = np.zeros((B, M, D), dtype=np.float32)
    for b in range(B):
        out[b] = res.results[2 * b]["y"] + res.results[2 * b + 1]["y"] + const
    return out, res


def kernel(**inputs) -> np.ndarray:
    out, _ = run(inputs, trace=False)
    return out


# revision 6
# speedup vs baseline: 1.0468x; 1.0326x over previous
"""Trainium2 Bass kernel for sparse causal attention (nn_CausalAttentionKV).

Reference computation (fp32, single device):
    q_all = x @ Wq + bq ; k_all = x @ Wk + bk ; v_all = x @ Wv + bv
    q = gather(q_all, query_idx)        # (B, M, D) selected query rows
    att = softmax(mask(q k^T / sqrt(hd)))   # per-query causal mask t <= qidx[m]
    y = (att v) @ Wo + bo

Shapes: B=4, T=4096, D=2048, n_head=16, hd=128, M=512.

Sharding (8 cores): core = 2*b + g  handles batch b and head-group g
(8 heads = 1024 feature cols).  Q/K/V projections are column-parallel,
out-proj is row-parallel; the two partial outputs per batch are summed
on the host.  All matmul inputs are bf16 (fp32 PSUM accumulation).

FUSED design (v3): attention is fused into the K/V projection pass,
flash-attention style.  For each 512-key window ts: project K (8 heads)
and V, then immediately compute scores, mask, exp and P@V for all heads
on those keys.  This hides the ~93us of scalar-engine exp work (which
previously bounded a separate attention phase) under the projection
matmul wall, eliminates the K^T DRAM round trip, and turns V into a
small SBUF ring instead of an 8 MB resident tensor.

Softmax bookkeeping avoids PE work: exp outputs accumulate into a
per-head fp32 e_total on the vector engine; ONE row-sum matmul per
head (vs one per chunk-pair) yields l; 1/l is broadcast across
partitions with a cheap bf16 rank-1 matmul (vs fp32 LOW_HIGH passes),
and those chains run DURING the last active window so normalized
outputs are ready the moment the last P@V lands.  The un-normalized
P@V partials accumulate into po_sb on the vector engine (PSUM holds
only one window's partial).

Startup: the Q projection is pipelined by d-chunk (8 PSUM banks
accumulate all 8 heads while wq/xq stream just-in-time) with wk DMA
chunks interleaved into the same stream, so the PE starts ~2us in and
window 0 is fed the moment AQ retires.  Phase C streams wo in 512-col
slices so it needs no resident footprint and starts as the tail norms
finish.
"""

import sys
import types
from contextlib import ExitStack

import numpy as np
import ml_dtypes

import concourse.bass as bass
import concourse.tile as tile
import concourse.mybir as mybir
from concourse import bacc
from concourse.bass_utils import run_bass_kernel_spmd

BF16 = mybir.dt.bfloat16
F32 = mybir.dt.float32
NPBF = ml_dtypes.bfloat16

B, T, D = 4, 4096, 2048
NH, HD, M = 16, 128, 512
NHG = 8            # heads per core (group)
DG = NHG * HD      # 1024 feature cols per core
NT = T // 128      # 32 t-chunks
ND = D // 128      # 16 d-chunks
KTS = 512          # keys per fused iteration
NTS = T // KTS     # 8 fused iterations
MASK_VAL = np.float32(-30000.0)


def _install_ntff_hook():
    """Register the axon NTFF profiling hook if the image's antenv lacks it."""
    try:
        from antenv.axon_hooks import get_axon_ntff_profile_hook  # noqa: F401
        return
    except ImportError:
        pass
    try:
        import antenv
        from trn_agent_boot.trn_boot import _ntff_profile_via_ctypes

        mod = types.ModuleType("antenv.axon_hooks")
        hook = [None]
        mod.set_axon_ntff_profile_hook = lambda h: hook.__setitem__(0, h)
        mod.get_axon_ntff_profile_hook = lambda: hook[0]
        sys.modules["antenv.axon_hooks"] = mod
        antenv.axon_hooks = mod
        mod.set_axon_ntff_profile_hook(
            _ntff_profile_via_ctypes("/opt/axon/libaxon_pjrt.so")
        )
    except Exception:
        pass


def build_program(flo, fhi):
    """Build the per-core Bass program.

    flo[i]: first m column with any allowed key in t-chunk i (cols below
    are fully masked there -> never computed).
    fhi[i]: first m column fully allowed in t-chunk i (cols beyond need
    no mask add).  Both are unions over the 4 batches so one program
    serves all cores.  flo is nondecreasing (qidx sorted per batch).
    """
    nc = bacc.Bacc("TRN2", target_bir_lowering=False, debug=False)

    xT = nc.dram_tensor("xT", [D, T], BF16, kind="ExternalInput")
    xqT = nc.dram_tensor("xqT", [D, M], BF16, kind="ExternalInput")
    wk = nc.dram_tensor("wk", [D, DG], BF16, kind="ExternalInput")
    wv = nc.dram_tensor("wv", [D, DG], BF16, kind="ExternalInput")
    wq = nc.dram_tensor("wq", [D, DG], BF16, kind="ExternalInput")
    wo = nc.dram_tensor("wo", [DG, D], BF16, kind="ExternalInput")
    maskd = nc.dram_tensor("mask", [T, M], BF16, kind="ExternalInput")
    bks = nc.dram_tensor("bks", [128, NHG], F32, kind="ExternalInput")
    bqs = nc.dram_tensor("bqs", [128, NHG], F32, kind="ExternalInput")
    y = nc.dram_tensor("y", [M, D], F32, kind="ExternalOutput")

    # (c*128+p, t) views for chunked DMA
    xTr = xT.rearrange("(c p) t -> p c t", p=128)
    xqTr = xqT.rearrange("(c p) t -> p c t", p=128)
    wkr = wk.rearrange("(c p) t -> p c t", p=128)
    wvr = wv.rearrange("(c p) t -> p c t", p=128)
    wqr = wq.rearrange("(c p) t -> p c t", p=128)
    wor = wo.rearrange("(c p) t -> p c t", p=128)
    maskr = maskd.rearrange("(c p) t -> p c t", p=128)

    # active chunks per ts window (flo nondecreasing -> consecutive prefix)
    def win_chunks(ts):
        return [i for i in range(4 * ts, 4 * ts + 4) if flo[i] < M]

    active_ts = [ts for ts in range(NTS) if win_chunks(ts)]
    last_ts = max(active_ts)
    mlo = [min(flo[4 * g : 4 * g + 4]) for g in range(NTS)]
    mhi = [max(fhi[4 * g : 4 * g + 4]) for g in range(NTS)]
    wmax = max(max(mhi[g] - mlo[g], 1) for g in range(NTS))

    with ExitStack() as ctx:
        tc = ctx.enter_context(tile.TileContext(nc))

        # ---- persistent tiles --------------------------------------
        persist = ctx.enter_context(tc.tile_pool(name="persist", bufs=1))
        # qt[j] holds Q^T until the last scores; the normalized output
        # O^T overwrites it in place at the tail (disjoint lifetimes).
        qt_t = [persist.tile([128, M], BF16, name=f"qt{j}", tag=f"qt{j}") for j in range(NHG)]
        ot_t = qt_t
        etot = [persist.tile([128, M], F32, name=f"et{j}", tag=f"et{j}") for j in range(NHG)]
        po_sb = [persist.tile([128, M], F32, name=f"po{j}", tag=f"po{j}") for j in range(NHG)]
        lb_t = [persist.tile([128, M], BF16, name=f"lb{j}", tag=f"lb{j}") for j in range(NHG)]
        bias_k = persist.tile([128, NHG], F32, name="bias_k", tag="bias_k")
        bias_q = persist.tile([128, NHG], F32, name="bias_q", tag="bias_q")
        zbias = persist.tile([128, 1], F32, name="zbias", tag="zbias")
        ones_c = persist.tile([128, 1], BF16, name="ones_c", tag="ones_c")
        ones_r = persist.tile([1, 128], BF16, name="ones_r", tag="ones_r")

        nc.sync.dma_start(bias_k[:], bks[:])
        nc.sync.dma_start(bias_q[:], bqs[:])
        nc.vector.memset(zbias[:], 0.0)
        nc.vector.memset(ones_c[:], 1.0)
        nc.vector.memset(ones_r[:], 1.0)

        # wk tiles must exist before AQ so its DMA rides the AQ stream
        wkp = ctx.enter_context(tc.tile_pool(name="wkp", bufs=1))
        wvp = ctx.enter_context(tc.tile_pool(name="wvp", bufs=1))
        wk_t = [wkp.tile([128, 4, DG], BF16, name=f"wk{d}", tag=f"wk{d}") for d in range(4)]
        wv_t = [wvp.tile([128, 4, DG], BF16, name=f"wv{d}", tag=f"wv{d}") for d in range(4)]

        # ---- phase AQ: Qt[j] = ((xq @ wq_j + bq_j)/sqrt(hd))^T -----
        # d-chunk pipelined: all 8 heads accumulate in 8 PSUM banks while
        # wq/xq stream just-in-time; wk chunks interleave into the same
        # sync-queue stream so window 0 is fed when AQ retires.
        with (
            nc.named_scope("phase_AQ"),
            tc.tile_pool(name="wqp", bufs=2) as wqp,
            tc.tile_pool(name="xqp", bufs=2) as xqp,
            tc.tile_pool(name="pq", bufs=1, space="PSUM") as pqp,
        ):
            pq = [pqp.tile([128, M], F32, name=f"pq{j}", tag=f"pq{j}") for j in range(NHG)]
            for d in range(ND):
                wq_d = wqp.tile([128, DG], BF16, name="wqd", tag="wqd")
                nc.sync.dma_start(wq_d[:], wqr[:, d, :])
                xq_d = xqp.tile([128, M], BF16, name="xqd", tag="xqd")
                nc.sync.dma_start(xq_d[:], xqTr[:, d, :])
                nc.sync.dma_start(wk_t[d // 4][:, d % 4, :], wkr[:, d, :])
                for j in range(NHG):
                    nc.tensor.matmul(
                        pq[j][:],
                        wq_d[:, j * 128 : (j + 1) * 128],
                        xq_d[:],
                        start=(d == 0),
                        stop=(d == ND - 1),
                        skip_group_check=True,
                    )
            inv_s = 1.0 / float(np.sqrt(HD))
            for j in range(NHG):
                nc.scalar.activation(
                    qt_t[j][:],
                    pq[j][:],
                    mybir.ActivationFunctionType.Identity,
                    scale=inv_s,
                    bias=bias_q[:, j : j + 1],
                )

        # wv after AQ's stream (needed ~27us into the fused pass)
        for d in range(4):
            nc.sync.dma_start(wv_t[d][:], wvr[:, 4 * d : 4 * d + 4, :])

        # ---- fused pass: K/V projection + attention per ts window --
        with (
            nc.named_scope("phase_F"),
            tc.tile_pool(name="xtp", bufs=2) as xtp,
            tc.tile_pool(name="ktp", bufs=3) as ktp,
            tc.tile_pool(name="vtp", bufs=5) as vtp,
            tc.tile_pool(name="esb", bufs=28) as esb,
            tc.tile_pool(name="mkp", bufs=3) as mkp,
            tc.tile_pool(name="ebf", bufs=2) as ebfp,
            tc.tile_pool(name="lsb", bufs=3) as lsbp,
            tc.tile_pool(name="kv", bufs=2, space="PSUM") as kvp,
            tc.tile_pool(name="ps", bufs=6, space="PSUM") as psp,
        ):
            et_started = [False] * NHG   # etot[j] initialized?
            po_started = [False] * NHG   # po_sb[j] initialized?

            def emit_scores(j, ts, kt, chunks, mk):
                """Scores+mask+exp for head j on window ts; per-chunk tiles."""
                work = []
                for i in chunks:
                    lo, hi = flo[i], fhi[i]
                    u = i % 4
                    pst = psp.tile([128, M], F32, name="pst", tag="ps")
                    nc.tensor.matmul(
                        pst[:, lo:M],
                        kt[:, u * 128 : (u + 1) * 128],
                        qt_t[j][:, lo:M],
                        start=True,
                        stop=True,
                        skip_group_check=True,
                    )
                    if lo < hi:
                        nc.vector.tensor_add(
                            pst[:, lo:hi],
                            pst[:, lo:hi],
                            mk[:, u, lo - mlo[ts] : hi - mlo[ts]],
                        )
                    e = esb.tile([128, M], BF16, name="e", tag="e")
                    nc.scalar.activation(
                        e[:, lo:M],
                        pst[:, lo:M],
                        mybir.ActivationFunctionType.Exp,
                        bias=zbias[:],
                    )
                    # accumulate softmax denominator on the DVE
                    if not et_started[j]:
                        nc.vector.tensor_copy(etot[j][:, lo:M], e[:, lo:M])
                        if lo > 0:
                            nc.vector.memset(etot[j][:, 0:lo], 0.0)
                        et_started[j] = True
                    else:
                        nc.vector.tensor_add(
                            etot[j][:, lo:M], etot[j][:, lo:M], e[:, lo:M]
                        )
                    work.append((i, e, lo))
                return work

            def emit_lchain(j):
                """l = rowsum(etot[j]); lb[j] = broadcast(1/l).

                Runs during the last active window; PSUM comes from the
                shared "ps" ring so no extra banks are needed.
                """
                ebf = ebfp.tile([128, M], BF16, name="ebf", tag="ebf")
                nc.vector.tensor_copy(ebf[:], etot[j][:])
                pl = psp.tile([128, M], F32, name="pl", tag="ps")
                nc.tensor.matmul(
                    pl[0:1, :], ones_c[:], ebf[:],
                    start=True, stop=True, skip_group_check=True,
                )
                l_sb = lsbp.tile([1, M], F32, name="l", tag="l")
                linv = lsbp.tile([1, M], F32, name="linv", tag="linv")
                linb = lsbp.tile([1, M], BF16, name="linb", tag="linb")
                nc.vector.tensor_copy(l_sb[:], pl[0:1, :])
                nc.vector.reciprocal_approx_fast(linv[:], l_sb[:])
                nc.vector.tensor_copy(linb[:], linv[:])
                pb = psp.tile([128, M], F32, name="pb", tag="ps")
                nc.tensor.matmul(
                    pb[:], ones_r[:], linb[:],
                    start=True, stop=True, skip_group_check=True,
                )
                nc.scalar.copy(lb_t[j][:], pb[:])

            def emit_pv(j, work, vts):
                """P@V for head j into PSUM, then DVE-accumulate to po_sb.

                PV matmuls run in chunk order (lo nondecreasing), so the
                start=True region [lo0:M] covers every later chunk's
                [lo_i:M] and no PSUM region is read uninitialized.
                """
                lo0 = work[0][2]
                pot = psp.tile([128, M], F32, name="pot", tag="ps")
                for k, (i, e, lo) in enumerate(work):
                    nc.tensor.matmul(
                        pot[:, lo:M],
                        vts[i % 4][:, j * 128 : (j + 1) * 128],
                        e[:, lo:M],
                        start=(k == 0),
                        stop=(k == len(work) - 1),
                        skip_group_check=True,
                    )
                if not po_started[j]:
                    nc.vector.tensor_copy(po_sb[j][:, lo0:M], pot[:, lo0:M])
                    if lo0 > 0:
                        nc.vector.memset(po_sb[j][:, 0:lo0], 0.0)
                    po_started[j] = True
                else:
                    nc.vector.tensor_add(
                        po_sb[j][:, lo0:M], po_sb[j][:, lo0:M], pot[:, lo0:M]
                    )

            for ts in range(NTS):
                chunks = win_chunks(ts)
                if not chunks:
                    continue
                xt_t = [xtp.tile([128, 4, KTS], BF16, name=f"xt{d}", tag=f"xt{d}") for d in range(4)]
                for d in range(4):
                    nc.sync.dma_start(
                        xt_t[d][:], xTr[:, 4 * d : 4 * d + 4, ts * KTS : (ts + 1) * KTS]
                    )
                mk = mkp.tile([128, 4, wmax], BF16, name="mk", tag="mk")
                if mlo[ts] < M and mhi[ts] > mlo[ts]:
                    nc.sync.dma_start(
                        mk[:, :, : mhi[ts] - mlo[ts]],
                        maskr[:, 4 * ts : 4 * ts + 4, mlo[ts] : mhi[ts]],
                    )
                # K projection, one head at a time; scores chase the K
                # stream; on the last window the l-chains ride along too.
                kts = {}
                sc_q = []
                for j in range(NHG):
                    pk = kvp.tile([128, KTS], F32, name="pk", tag="kv")
                    for d in range(ND):
                        nc.tensor.matmul(
                            pk[:],
                            wk_t[d // 4][:, d % 4, j * 128 : (j + 1) * 128],
                            xt_t[d // 4][:, d % 4, :],
                            start=(d == 0),
                            stop=(d == ND - 1),
                        )
                    kt = ktp.tile([128, KTS], BF16, name="kt", tag="kt")
                    nc.scalar.activation(
                        kt[:], pk[:],
                        mybir.ActivationFunctionType.Identity,
                        bias=bias_k[:, j : j + 1],
                    )
                    kts[j] = kt
                    if j >= 1:
                        sc_q.append((j - 1, emit_scores(j - 1, ts, kts.pop(j - 1), chunks, mk)))
                        if ts == last_ts:
                            emit_lchain(j - 1)
                # V projection: 4 v tiles [t=128, DG] per window
                vts = []
                for u in range(4):
                    vt = vtp.tile([128, DG], BF16, name="vt", tag="vt")
                    for f in range(2):
                        pv = kvp.tile([128, 512], F32, name="pv", tag="kv")
                        for d in range(ND):
                            nc.tensor.matmul(
                                pv[:],
                                xt_t[d // 4][:, d % 4, u * 128 : (u + 1) * 128],
                                wv_t[d // 4][:, d % 4, f * 512 : (f + 1) * 512],
                                start=(d == 0),
                                stop=(d == ND - 1),
                            )
                        nc.vector.tensor_copy(vt[:, f * 512 : (f + 1) * 512], pv[:])
                    vts.append(vt)
                # head 7's scores go after the V stream: its exp may wait on
                # an e-ring slot freed by PV(j0), whose u=3 matmul needs the
                # u=3 v-copy -- so no DVE work of head 7 may precede vcopy(u3)
                sc_q.append((NHG - 1, emit_scores(NHG - 1, ts, kts.pop(NHG - 1), chunks, mk)))
                # this window's P@V; chunk order defers the u=3 dependency
                for j, work in sc_q:
                    emit_pv(j, work, vts)

            # head 7's l-chain must come after the last PV drain: inline it
            # would put a PE matmul (pb) behind DVE work that waits on the
            # final window's exps, whose e-ring slots only free at PV time.
            emit_lchain(NHG - 1)
            # normalized transposed output overwrites qt in place
            for j in range(NHG):
                nc.vector.tensor_mul(ot_t[j][:], po_sb[j][:], lb_t[j][:])

        # ---- phase C: y = O @ wo, wo streamed in 512-col slices ----
        with (
            nc.named_scope("phase_C"),
            tc.tile_pool(name="wop", bufs=2) as wop,
            tc.tile_pool(name="py", bufs=3, space="PSUM") as pyp,
            tc.tile_pool(name="ysb", bufs=3) as ysb,
        ):
            for fo in range(D // 512):
                wo_s = wop.tile([128, NHG, 512], BF16, name="wos", tag="wos")
                nc.sync.dma_start(
                    wo_s[:], wor[:, :, fo * 512 : (fo + 1) * 512]
                )
                for mb in range(M // 128):
                    py = pyp.tile([128, 512], F32, name="py", tag="py")
                    for j in range(NHG):
                        nc.tensor.matmul(
                            py[:],
                            ot_t[j][:, mb * 128 : (mb + 1) * 128],
                            wo_s[:, j, :],
                            start=(j == 0),
                            stop=(j == NHG - 1),
                        )
                    ys = ysb.tile([128, 512], F32, name="ys", tag="ys")
                    nc.scalar.copy(ys[:], py[:])
                    nc.sync.dma_start(
                        y[
                            mb * 128 : (mb + 1) * 128,
                            fo * 512 : (fo + 1) * 512,
                        ],
                        ys[:],
                    )

    nc.compile()
    return nc


_cache = {}


def _get_program(flo, fhi):
    key = (tuple(flo), tuple(fhi))
    if key not in _cache:
        _cache[key] = build_program(list(flo), list(fhi))
    return _cache[key]


def _prep(inputs):
    x = np.asarray(inputs["x"], dtype=np.float32)
    qidx = np.asarray(inputs["query_idx"]).astype(np.int64)
    Wq = np.asarray(inputs["Wq"], dtype=np.float32)
    Wk = np.asarray(inputs["Wk"], dtype=np.float32)
    Wv = np.asarray(inputs["Wv"], dtype=np.float32)
    Wo = np.asarray(inputs["Wo"], dtype=np.float32)
    bq = np.asarray(inputs["bq"], dtype=np.float32)
    bk = np.asarray(inputs["bk"], dtype=np.float32)
    bv = np.asarray(inputs["bv"], dtype=np.float32)
    bo = np.asarray(inputs["bo"], dtype=np.float32)

    # Per-t-chunk skip bounds, union over batches.  flo[i] = first m that
    # attends into chunk i (everything below is fully masked there);
    # fhi[i] = one past the last m only partially covered by chunk i.
    # Computed positionally so they are correct even for unsorted
    # query_idx (just less effective at skipping).
    flo = [M] * NT
    fhi = [0] * NT
    for b in range(B):
        for i in range(NT):
            allowed = qidx[b] >= 128 * i          # chunk i not fully masked
            partial = qidx[b] < 128 * (i + 1)     # chunk i not fully allowed
            lo_b = int(np.argmax(allowed)) if allowed.any() else M
            hi_b = M - int(np.argmax(partial[::-1])) if partial.any() else 0
            flo[i] = min(flo[i], lo_b)
            fhi[i] = max(fhi[i], hi_b)

    in_maps = []
    tgrid = np.arange(T)[:, None]
    for core in range(8):
        b, g = divmod(core, 2)
        sl = slice(g * DG, (g + 1) * DG)
        xb = x[b]
        mask = np.where(tgrid <= qidx[b][None, :], np.float32(0), MASK_VAL)
        in_maps.append(
            {
                "xT": np.ascontiguousarray(xb.T.astype(NPBF)),
                "xqT": np.ascontiguousarray(xb[qidx[b]].T.astype(NPBF)),
                "wk": np.ascontiguousarray(Wk[:, sl].astype(NPBF)),
                "wv": np.ascontiguousarray(Wv[:, sl].astype(NPBF)),
                "wq": np.ascontiguousarray(Wq[:, sl].astype(NPBF)),
                "wo": np.ascontiguousarray(Wo[sl, :].astype(NPBF)),
                "mask": np.ascontiguousarray(mask.astype(NPBF)),
                "bks": np.ascontiguousarray(bk[sl].reshape(NHG, 128).T),
                "bqs": np.ascontiguousarray(
                    (bq[sl] / np.sqrt(HD)).reshape(NHG, 128).T.astype(np.float32)
                ),
            }
        )

    const = (bv.astype(np.float64) @ Wo.astype(np.float64) + bo).astype(np.float32)
    return flo, fhi, in_maps, const


def run(inputs, trace=False, trace_kwargs=None):
    _install_ntff_hook()
    flo, fhi, in_maps, const = _prep(inputs)
    nc = _get_program(flo, fhi)
    res = run_bass_kernel_spmd(
        nc, in_maps, list(range(8)), trace=trace, **(trace_kwargs or {})
    )
    out = np.zeros((B, M, D), dtype=np.float32)
    for b in range(B):
        out[b] = res.results[2 * b]["y"] + res.results[2 * b + 1]["y"] + const
    return out, res


def kernel(**inputs) -> np.ndarray:
    out, _ = run(inputs, trace=False)
    return out
